# revision 5
# baseline (speedup 1.0000x reference)
"""Trainium2 Bass kernel for nn_DAttention:
out[b,c,d,h,w] = x[b,c,d,h,w] * mean_{c,h,w}(x[b,:,d,:,:]).

Sharding: pure data parallel over batch B=8 -> one batch per NeuronCore.
Numerics: HBM I/O in bf16 (host converts f32->bf16 in, bf16->f32 out);
the mean is accumulated in fp32, so element error is ~2 bf16 roundings
(~0.4%) -- far inside the 2e-2 gate. 32 MiB in + 32 MiB out per core.

DMA load-balancing: SDMA engine 15 (serving SBUF partitions 92-95 and
124-127 via the port swizzle port = bits[4:2]<<1|bit[6]) runs ~14%
slower than the other 15 engines under load (fat-tailed packet
durations; HW arbitration) and is the critical path of a uniform
[128, F] layout. So each 2^19-element d-slice is dealt (on the host)
into a main tile [128, 3196] plus an extra tile [120, 960] that lives
on partitions 0-119 only: engines 13 and 15 (ports of partitions
120-127) carry ~11.5% fewer bytes, which absorbs engine 15's deficit.
All DMAs stay rectangular; no pad elements anywhere.

Per-slice schedule (engines balanced so the post-load tail stays
store-DMA-paced):
  ACT: activation-Copy (into dead PSUM scratch) with accum_out ->
       fp32 column sums of xt[:, :A] and of the extra tile
  DVE: tensor_reduce(add) -> fp32 column sums of xt[:, A:]
  PE : three accumulated fp32 matmuls against a constant 128x128
       matrix of 1/2^19 -> total sum broadcast to [128,1] PSUM
  ACT: tiny copy mean PSUM->SBUF
  DVE: two tensor_scalar multiplies (bf16 2x mode) -> out tiles
  DMA: main loads + extra loads + extra stores on the SP HWDGE ring,
       main stores on the ACT ring; loads issued LOOKAHEAD slices early
"""
import numpy as np
import ml_dtypes

import concourse.bacc as bacc
import concourse.tile as tile
import concourse.mybir as mybir
from concourse.bass_utils import run_bass_kernel_spmd

BF16 = ml_dtypes.bfloat16

B, C, D, H, W = 8, 32, 32, 128, 128
N = C * H * W           # 524288 = 2**19 elements per (b, d) slice
RECIP = 1.0 / N         # exact in fp32
FE = 3196               # main tile free size (128 partitions)
FX = 960                # extra tile free size (120 partitions)
PX = 120
L120 = PX * (FE + FX)   # 498720 elems dealt to partitions 0-119
assert 128 * FE + PX * FX == N
A_SPLIT = 2048          # ACT reduces xt[:, :A], DVE reduces xt[:, A:]
LOOKAHEAD = 3

_NC = None


def _build_nc(xin_bufs=8, out_bufs=4):
    nc = bacc.Bacc("TRN2", target_bir_lowering=False, debug=False)
    xm = nc.dram_tensor("xm", [D, 128, FE], mybir.dt.bfloat16, kind="ExternalInput")
    xe = nc.dram_tensor("xe", [D, PX, FX], mybir.dt.bfloat16, kind="ExternalInput")
    om = nc.dram_tensor("om", [D, 128, FE], mybir.dt.bfloat16, kind="ExternalOutput")
    oe = nc.dram_tensor("oe", [D, PX, FX], mybir.dt.bfloat16, kind="ExternalOutput")
    with tile.TileContext(nc) as tc:
        with (
            tc.tile_pool(name="xin", bufs=xin_bufs) as xpool,
            tc.tile_pool(name="xein", bufs=xin_bufs) as xepool,
            tc.tile_pool(name="oout", bufs=out_bufs) as opool,
            tc.tile_pool(name="oeout", bufs=out_bufs) as oepool,
            tc.tile_pool(name="small", bufs=6) as spool,
            tc.tile_pool(name="psum", bufs=2, space="PSUM") as ppool,
            tc.tile_pool(name="psc", bufs=1, space="PSUM") as scpool,
            tc.tile_pool(name="const", bufs=1) as cpool,
        ):
            recip = cpool.tile([128, 128], mybir.dt.float32)
            nc.gpsimd.memset(recip[:], RECIP)

            xts, xets = {}, {}

            def issue_loads(d):
                xt = xpool.tile([128, FE], mybir.dt.bfloat16, tag="xt")
                nc.sync.dma_start(xt[:], xm[d])
                xet = xepool.tile([PX, FX], mybir.dt.bfloat16, tag="xet")
                nc.sync.dma_start(xet[:], xe[d])
                xts[d], xets[d] = xt, xet

            for d in range(LOOKAHEAD):
                issue_loads(d)
            for d in range(D):
                xt, xet = xts.pop(d), xets.pop(d)
                csa = spool.tile([128, 1], mybir.dt.float32, tag="csa")
                csb = spool.tile([128, 1], mybir.dt.float32, tag="csb")
                csd = spool.tile([PX, 1], mybir.dt.float32, tag="csd")
                scrA = scpool.tile([128, A_SPLIT], mybir.dt.float32, tag="scA")
                scrE = scpool.tile([PX, FX], mybir.dt.float32, tag="scE")
                nc.scalar.activation(
                    scrA[:], xt[:, :A_SPLIT],
                    mybir.ActivationFunctionType.Copy, accum_out=csa[:],
                )
                nc.scalar.activation(
                    scrE[:], xet[:],
                    mybir.ActivationFunctionType.Copy, accum_out=csd[:],
                )
                nc.vector.tensor_reduce(
                    csb[:], xt[:, A_SPLIT:],
                    mybir.AxisListType.X, mybir.AluOpType.add,
                )
                dv = ppool.tile([128, 1], mybir.dt.float32, tag="dv")
                nc.tensor.matmul(dv[:], recip[:], csa[:], start=True, stop=False)
                nc.tensor.matmul(dv[:], recip[:], csb[:], start=False, stop=False)
                nc.tensor.matmul(dv[:], recip[:PX, :], csd[:], start=False, stop=True)
                dvs = spool.tile([128, 1], mybir.dt.float32, tag="dvs")
                nc.scalar.copy(dvs[:], dv[:])
                ot = opool.tile([128, FE], mybir.dt.bfloat16, tag="ot")
                nc.vector.tensor_scalar_mul(ot[:], xt[:], dvs[:])
                oet = oepool.tile([PX, FX], mybir.dt.bfloat16, tag="oet")
                nc.vector.tensor_scalar_mul(oet[:], xet[:], dvs[:PX])
                if d + LOOKAHEAD < D:
                    issue_loads(d + LOOKAHEAD)
                nc.scalar.dma_start(om[d], ot[:])
                nc.sync.dma_start(oe[d], oet[:])
    nc.compile()
    return nc


def _get_nc():
    global _NC
    if _NC is None:
        _NC = _build_nc()
    return _NC


def _deal_in(xb_core: np.ndarray):
    """[C,D,H,W] f32 -> (xm [D,128,FE], xe [D,PX,FX]) bf16."""
    xd = np.ascontiguousarray(xb_core.astype(BF16).transpose(1, 0, 2, 3)).reshape(D, N)
    A = xd[:, :L120].reshape(D, PX, FE + FX)
    xm = np.empty((D, 128, FE), BF16)
    xm[:, :PX] = A[:, :, :FE]
    xm[:, PX:] = xd[:, L120:].reshape(D, 8, FE)
    return xm, np.ascontiguousarray(A[:, :, FE:])


def _deal_out(om_core: np.ndarray, oe_core: np.ndarray):
    """(om [D,128,FE], oe [D,PX,FX]) bf16 -> [C,D,H,W] f32."""
    Bm = np.empty((D, PX, FE + FX), BF16)
    Bm[:, :, :FE] = om_core[:, :PX]
    Bm[:, :, FE:] = oe_core
    od = np.empty((D, N), BF16)
    od[:, :L120] = Bm.reshape(D, -1)
    od[:, L120:] = om_core[:, PX:].reshape(D, -1)
    return od.reshape(D, C, H, W).transpose(1, 0, 2, 3).astype(np.float32)


def run(x: np.ndarray, trace: bool = False, tmpdir: str | None = None):
    """Run on 8 NeuronCores; returns (out, BassKernelResults)."""
    x = np.asarray(x)
    assert x.shape == (B, C, D, H, W), x.shape
    nc = _get_nc()
    in_maps = []
    for b in range(B):
        xm, xe_ = _deal_in(x[b])
        in_maps.append({"xm": xm, "xe": xe_})
    res = run_bass_kernel_spmd(
        nc, in_maps, core_ids=list(range(B)), trace=trace, tmpdir=tmpdir
    )
    out = np.stack([_deal_out(r["om"], r["oe"]) for r in res.results])
    return out, res


def kernel(x: np.ndarray) -> np.ndarray:
    out, _ = run(x)
    return out
